# revision 25
# baseline (speedup 1.0000x reference)
"""Stereo correlation cost volume kernel for Trainium2 (8 NeuronCores).

  out[b, d, h, w] = mean_c( L[b,c,h,w] * R[b,c,h,w-d] )  for w >= d, else 0
  B=8, C=64, H=128, W=256, D=64.

Sharding: data-parallel over batch; core b handles batch b.

Host-side packing keeps every DMA a 2-dim, 128-partition, long-contiguous
transfer (the DMA cost model rewards >=512B runs and all 128 partitions):
  - lr  [128 (j c), NG*2048] fp16: partition j*64+c holds rows h=8g+4j+hp,
    L pre-scaled by 1/C (folds the channel mean into the matmul).
  - out [128 (j d), NG*1024 + 64] fp16: partition j*64+d holds, per group,
    the skewed diagonal run out[d, h0+4j.., w in [d, d+256)] as 1024
    contiguous elements; chunk spills land in the next chunk's w < d zero
    zone (spilled values are exact zeros) or the 64-el per-row gap. The
    host postprocess un-skews with a vectorized gather.

Per-core pipeline, per group of NH=8 rows h (j = hh//4):
  1. One [128, 2048] load DMA per group.
  2. PE Gram G2[u, w] = sum_c R[c,u] * (L[c,w]/C), two trimmed blocks per
     h (A: u<128 x w<192, B: u,w in [128,256)), batched 4h per PSUM pair.
  3. ACT/DVE copy the Gram band PSUM->SBUF fp16 into g8 (a->ACT, b split).
  4. Skew T3[u, d] = G2[u, u+d] fully on-chip: gpsimd local_scatter with a
     per-partition index table (idx[p, w] = w-p in [0,64), else -1), one
     fused A+B call per h. Out-of-band entries stay exact zeros.
  5. PE transposes T3 -> PT[d, u] per 2h into a PSUM bank (j selects the
     partition half); DVE fp16-copies PT into s8 [128 (j d), 1024].
  6. One [128, 1024] out DMA per group (consume of group g-1 is woven
     between group g's produce windows to keep all five engines busy).
A DRAM-scratch skew path (skew-via-strided-reread) remains available via
K_POOL < 8 but is disabled: the local_scatter path is strictly cheaper.
The runner pre-zeros output buffers, so untouched zones stay zero.
"""

import os
import sys

import numpy as np

sys.path.insert(0, "/opt/trn_rl_repo")

import ml_dtypes  # noqa: E402,F401

import concourse.bass as bass  # noqa: E402,F401
import concourse.bacc as bacc  # noqa: E402
import concourse.mybir as mybir  # noqa: E402
from concourse.bass import AP  # noqa: E402
from concourse.bass_utils import run_bass_kernel_spmd  # noqa: E402
from concourse.masks import make_identity  # noqa: E402
from concourse.tile import TileContext  # noqa: E402

B, C, H, W = 8, 64, 128, 256
D = 64
NH = 8  # h rows per group
NG = H // NH  # 16 groups
K_POOL = 8  # rows per group skewed on gpsimd instead of DRAM scratch
NS = NH - K_POOL  # rows per group using the scratch path
F32 = mybir.dt.float32
F16 = mybir.dt.float16
I16 = mybir.dt.int16

# compute/in/scratch dtype: "bf16" (fast) or "f32" (exact)
USE_BF16 = os.environ.get("CORVOL_F32", "") != "1"

# scratch layout (elements of the scratch dtype), per scratch h slot:
#   [A rows: 128 x 192][B rows: 128 x (128 data + 64 zero tail)]
# B's zero tail makes the (pitch+1)-strided re-read pull zeros for
# p+d >= 128, which the out write chains into the w < d zero zones.
A_COLS = 192
B_COLS = 128
B_PITCH = 192
BLKA = 128 * A_COLS
BLKB = 128 * B_PITCH
HSLOT = BLKA + BLKB
SCR_SLOTS = max(NS, 1) * (NG // 2)  # per-parity scratch slots
SCR_SIZE = SCR_SLOTS * HSLOT

GW = 320  # g8 cols per h: A band [0,192) + B band [192,320)

_CACHE = {}


def build():
    in_dt = F16 if USE_BF16 else F32
    out_dt = F16 if USE_BF16 else F32
    nc = bacc.Bacc()
    # lr packed host-side as [128 (j c), NG*2048]: partition j*64+c holds
    # rows h = 8g+4j+hp, [L | R] interleaved per row — every group load is
    # one [128, 2048] DMA with 4 KiB contiguous per partition.
    lr_dram = nc.dram_tensor(
        "lr", [128, NG * 4 * 2 * W], in_dt, kind="ExternalInput"
    )
    idx_dram = nc.dram_tensor("idx", [128, GW], I16, kind="ExternalInput")
    # out packed as [128 (j d), NG*1024 + 64]: partition j*64+d holds, per
    # group, the skewed diagonal run out[d, h0+4j.., w in [d, d+256)] as
    # 1024 contiguous elements; the trailing spill of each chunk lands
    # either in the next chunk's w < d zero zone (spilled values are exact
    # zeros by construction) or in the 64-el per-row gap. Host re-gathers.
    OROW = NG * 1024 + 64
    out_dram = nc.dram_tensor(
        "out", [128 * OROW], out_dt, kind="ExternalOutput"
    )
    scr = [
        nc.dram_tensor(f"scratch{i}", [SCR_SIZE], in_dt, kind="Internal")
        for i in range(2)
    ] if NS else [None, None]

    with TileContext(nc) as tc:
        with (
            tc.tile_pool(name="const", bufs=1) as pconst,
            tc.tile_pool(name="inp", bufs=16) as pin,
            tc.tile_pool(name="gband", bufs=6) as pg,
            tc.tile_pool(name="skew", bufs=6) as pt3,
            tc.tile_pool(name="outs", bufs=6) as ps8,
            tc.tile_pool(name="psA", bufs=2, space="PSUM") as ppa,
            tc.tile_pool(name="psB", bufs=2, space="PSUM") as ppb,
            tc.tile_pool(name="psT", bufs=2, space="PSUM") as ppt,
        ):
            ident = pconst.tile([128, 128], in_dt)
            make_identity(nc, ident)
            idx = pconst.tile([128, GW], I16)
            if NS:
                zeros = pconst.tile([128, SCR_SLOTS * 64], in_dt)
                nc.gpsimd.memset(zeros, 0.0)
                zv = zeros.rearrange("p (s e) -> p s e", s=SCR_SLOTS)
                # zero every B row's 64-el tail so the strided skew read
                # returns exact zeros for p+d >= 128 (out's w < d zones)
                for i in range(2):
                    nc.sync.dma_start(
                        out=AP(
                            scr[i],
                            BLKA + B_COLS,
                            [[B_PITCH, 128], [HSLOT, SCR_SLOTS], [1, 64]],
                        ),
                        in_=zv,
                    )
            # warmup: absorb the gpsimd ident-write wait on PE once
            scrap0 = ppa.tile([64, 64], in_dt, tag="g2a")
            nc.tensor.transpose(
                scrap0[0:1, :], ident[0:64, 0:1], ident[0:64, 0:64]
            )

            pending = None
            for g in range(NG):
                h0 = g * NH
                sbase = (g // 2) * NS * HSLOT
                st = scr[g % 2]

                # stage 1: group input load, 128 partitions = (j, c)
                lr8 = pin.tile([C * 2, 4 * 2 * W], in_dt, tag="lr8")
                if g == 0:
                    # split the first load so matmuls start sooner; slot
                    # the idx load (needed later, by the scatters) between
                    nc.sync.dma_start(
                        out=lr8[:, 0:1024], in_=lr_dram[:, 0:1024]
                    )
                    nc.sync.dma_start(out=idx, in_=idx_dram[:, :])
                    nc.sync.dma_start(
                        out=lr8[:, 1024:2048], in_=lr_dram[:, 1024:2048]
                    )
                else:
                    nc.sync.dma_start(
                        out=lr8, in_=lr_dram[:, g * 2048 : (g + 1) * 2048]
                    )

                # stages 2+3 interleaved: consume windows of the previous
                # group woven between this group's produce windows, so no
                # engine's program order serializes the wave
                s8p = None
                if pending is not None:
                    s8p = ps8.tile([128, 4 * 256], out_dt, tag="s8")
                    prev = pending

                g8 = pg.tile([128, NH * GW], in_dt, tag="g8")
                g8v = g8.rearrange("p (h c) -> p h c", h=NH)
                t3 = pt3.tile([128, NH * 128], in_dt, tag="t3")
                t3v = t3.rearrange("p (h d) -> p h d", h=NH)
                for i4 in range(NH // 4):
                    if s8p is not None:
                        _consume_window(
                            nc, prev, ident, ppt, s8p, in_dt, i4
                        )
                    hh0 = 4 * i4
                    g2a = ppa.tile([128, 4 * 256], F32, tag="g2a")
                    g2b = ppb.tile([128, 4 * B_COLS], F32, tag="g2b")
                    for q in range(4):
                        hh = hh0 + q
                        j, hp = hh // 4, hh % 4
                        p0 = j * C
                        Lv = lr8[p0 : p0 + C, hp * 512 : hp * 512 + W]
                        Rv = lr8[p0 : p0 + C, hp * 512 + W : hp * 512 + 2 * W]
                        nc.tensor.matmul(
                            g2a[:, q * 256 : q * 256 + A_COLS],
                            lhsT=Rv[:, 0:128],
                            rhs=Lv[:, 0:A_COLS],
                        )
                        nc.tensor.matmul(
                            g2b[:, q * B_COLS : (q + 1) * B_COLS],
                            lhsT=Rv[:, 128:256],
                            rhs=Lv[:, 128:256],
                        )
                    ga = g2a.rearrange("p (q c) -> p q c", q=4)[:, :, 0:A_COLS]
                    gb = g2b.rearrange("p (q c) -> p q c", q=4)
                    # f32-source copies batched per 4h; a -> ACT, b split
                    nc.scalar.copy(
                        g8v[:, hh0 : hh0 + 4, 0:A_COLS], ga
                    )
                    to_dve = (g + i4) % 2 == 0
                    beng = nc.vector.tensor_copy if to_dve else nc.scalar.copy
                    beng(g8v[:, hh0 : hh0 + 4, A_COLS:GW], gb[:, :, :])

                # stage 4a: on-chip skew, woven with remaining consume
                for hh in range(K_POOL):
                    if s8p is not None and hh < 2:
                        _consume_window(
                            nc, prev, ident, ppt, s8p, in_dt, 2 + hh
                        )
                    nc.gpsimd.local_scatter(
                        t3v[:, hh, :],
                        g8v[:, hh, :],
                        idx[:, :],
                        channels=128,
                        num_elems=128,
                        num_idxs=GW,
                    )

                # stage 4b: out-DMA of previous group [SP, before Gwrites]
                if s8p is not None:
                    _out_dma(nc, out_dram, s8p, g - 1)

                # stage 5: Gram band -> scratch for rows [K_POOL, NH)
                if NS:
                    nc.sync.dma_start(
                    out=AP(
                        st,
                        sbase,
                        [[A_COLS, 128], [HSLOT, NS], [1, A_COLS]],
                    ),
                        in_=g8v[:, K_POOL:NH, 0:A_COLS],
                    )
                    nc.sync.dma_start(
                        out=AP(
                            st,
                            sbase + BLKA,
                            [[B_PITCH, 128], [HSLOT, NS], [1, B_COLS]],
                        ),
                        in_=g8v[:, K_POOL:NH, A_COLS:GW],
                    )

                    # stage 6: skewed re-read [SP]
                    nc.sync.dma_start(
                        out=t3v[:, K_POOL:NH, 0:64],
                        in_=AP(
                            st,
                            sbase,
                            [[A_COLS + 1, 128], [HSLOT, NS], [1, 64]],
                        ),
                    )
                    nc.sync.dma_start(
                        out=t3v[:, K_POOL:NH, 64:128],
                        in_=AP(
                            st,
                            sbase + BLKA,
                            [[B_PITCH + 1, 128], [HSLOT, NS], [1, 64]],
                        ),
                    )
                pending = t3v

            # drain last group
            s8p = _consume(nc, pending, ident, ppt, ps8, in_dt, out_dt)
            _out_dma(nc, out_dram, s8p, NG - 1)
    nc.finalize()
    return nc


def _consume_window(nc, t3v, ident, ppt, s8, in_dt, i2):
    """One 2h window: transpose T3 -> PT (PSUM), fp16-copy into s8."""
    hh0 = 2 * i2
    j = hh0 // 4
    p0 = j * 64
    pt = ppt.tile([128, 512], in_dt, tag="pt")
    for q in range(2):
        hh = hh0 + q
        nc.tensor.transpose(
            pt[p0 : p0 + 64, q * 256 : q * 256 + 128],
            t3v[:, hh, 0:64],
            ident,
        )
        nc.tensor.transpose(
            pt[p0 : p0 + 64, q * 256 + 128 : q * 256 + 256],
            t3v[:, hh, 64:128],
            ident,
        )
    c0 = (hh0 % 4) * 256
    # 1/C folded into host-side L pre-scale; pure fp16 copy is DVE-fast
    nc.vector.tensor_copy(
        s8[p0 : p0 + 64, c0 : c0 + 512], pt[p0 : p0 + 64, :]
    )


def _consume(nc, t3v, ident, ppt, ps8, in_dt, out_dt):
    s8 = ps8.tile([128, 4 * 256], out_dt, tag="s8")
    for i2 in range(NH // 2):
        _consume_window(nc, t3v, ident, ppt, s8, in_dt, i2)
    return s8


def _out_dma(nc, out_dram, s8, g):
    # partition r = j*64+d writes its group-g chunk: 1024 contiguous els
    # at r*OROW + g*1024 (OROW = NG*1024 + 64; spill -> gap or zero zone)
    nc.sync.dma_start(
        out=AP(out_dram, g * 1024, [[NG * 1024 + 64, 128], [1, 1024]]),
        in_=s8,
    )


def _idx_table():
    p = np.arange(128)[:, None]
    w = np.arange(GW)[None, :]
    da = w - p
    db = (w - A_COLS) - p
    idx = np.where((w < A_COLS) & (da >= 0) & (da < 64), da, -1)
    idx = np.where((w >= A_COLS) & (db >= 0) & (db < 64), 64 + db, idx)
    return idx.astype(np.int16)


def make_in_map(left_b, right_b):
    """Host-side input prep: pack [C,H,2,W] -> [128 (j c), NG*2048]."""
    np_dt = np.float16 if USE_BF16 else np.float32
    # fold the 1/C mean into L so the device never scales
    lr = np.stack([left_b * (1.0 / C), right_b], axis=2).astype(np_dt)
    # h = 8g + 4j + hp  ->  [j, c, g, hp, t, w]
    lr = lr.reshape(C, NG, 2, 4, 2, W).transpose(2, 0, 1, 3, 4, 5)
    lr = np.ascontiguousarray(lr).reshape(128, NG * 4 * 2 * W)
    return {"lr": lr, "idx": _idx_table()}


_DECODE = {}


def postprocess(out_arr):
    """Host-side decode: un-skew [128 (j d), NG, 1024] -> [D, H, W] fp32."""
    a = np.asarray(out_arr).reshape(128, NG * 1024 + 64)[:, : NG * 1024]
    a = a.reshape(2, 64, NG, 1024)  # [j, d, g, k]
    if "idx" not in _DECODE:
        d_ = np.arange(64)[:, None, None]
        hp = np.arange(4)[None, :, None]
        w_ = np.arange(W)[None, None, :]
        k = hp * 256 + w_ - d_  # [64, 4, 256]
        _DECODE["mask"] = (k >= 0).astype(np.float32)
        _DECODE["idx"] = np.clip(k, 0, 1023).reshape(1, 64, 1, 1024)
    g = np.take_along_axis(a, _DECODE["idx"], axis=3).astype(np.float32)
    g = g.reshape(2, 64, NG, 4, 256) * _DECODE["mask"][None, :, None, :, :]
    # [j, d, g, hp, w] -> [d, (g, j, hp) = h, w]
    return np.ascontiguousarray(
        g.transpose(1, 2, 0, 3, 4).reshape(D, H, W)
    )


def kernel(left_feature, right_feature, max_disp):
    assert int(max_disp) == D
    left = np.asarray(left_feature, dtype=np.float32)
    right = np.asarray(right_feature, dtype=np.float32)
    assert left.shape == (B, C, H, W) and right.shape == (B, C, H, W)

    if "nc" not in _CACHE:
        _CACHE["nc"] = build()
    nc = _CACHE["nc"]

    in_maps = [make_in_map(left[b], right[b]) for b in range(B)]
    res = run_bass_kernel_spmd(nc, in_maps, list(range(B)))
    _CACHE["last_results"] = res
    out = np.stack([postprocess(res.results[b]["out"]) for b in range(B)], axis=0)
    return out
